# revision 1
# baseline (speedup 1.0000x reference)
"""Trainium2 Bass kernel for CustomMultiHeadAttention.

Problem: x[2,2048,1024], 16 heads, Dh=64. y = MHA(x) with Q/K/V/O projections.

Sharding (8 cores, no collectives):
  core c -> batch b = c//4, head-quarter hq = c%4 (4 heads, 256 model cols).
  Each core computes Q,K,V projections for its 4 heads over the FULL batch
  sequence, attention for those heads, and a PARTIAL o_proj (its 256 rows of
  Wo). The host sums the 4 bf16 partials per batch in fp32 and adds
  bo + bv@Wo (bv folds out of attention since softmax rows sum to 1).

Projections (fp8 DoubleRow residual):
  x and Wq/Wk/Wv ship as fp8e4m3 (hi, lo) pairs with lo = fp8(t - hi), the
  weights pre-scaled x16 to clear the fp8 subnormal range. Each projection
  sums the 3 cross terms hi*whi + hi*wlo + lo*whi with DoubleRow matmuls
  (2 k-tiles contracted per instruction): 12 matmuls per [128,512] output
  chunk at 0.5 cycles/row -- 4x cheaper than bf16 -- at ~bf16 accuracy.
  The x16 scales cancel: exp uses scale 0.125/256, and Wo ships as Wo/16.

Attention dataflow:
  K^T,Q^T  [256, 2048] bf16 in SBUF (x16), bias-added on DVE
  V        [2048, 4, 65] bf16 (x16), ones col 64 so AV emits sumexp
  S^T      per (head, kt, qt512): lhsT=kT[64,kt128], rhs=qT[64,qt512]
  P^T      = exp(S^T/2048) on ACT in [P,2,512] chunks -> exp_sb bf16
  O        per (head, qs128): out[q,0:65] = P^T[k,q].T @ [V|1], accumulated
           over 16 kt, one full PSUM bank per accumulation group (start=True
           resets the bank -- concurrent groups must never share one).
           q lands on partitions, so sumexp is a per-partition scalar and
           normalization is one DVE reciprocal + tensor_scalar per qs.
  oT       = PE transpose (identity matmul, bf16 PSUM out) + DVE copy
  y        = oT[256,q128].T @ Wo/16 -> [2048,1024] partial, staged to SBUF
           bf16 and DMA'd out.

Schedule: two head-pair passes (heads 0/1 for all qt, then 2/3) so the mt=1
K/Q projections move out of the congested early phase; QK+exp groups are
emitted 2 ahead and AV consumption lags 3 behind (the exp pool absorbs the
slack) -- every tensor is emitted strictly after its producers, which is
what Tile's program-order dependency tracking requires. The last qt's
o_proj runs right after the final exp on the freed psQ ring (all four AV
accumulators in parallel banks) with copies on the then-idle ACT engine.

PSUM (8 banks): psQ 2x[P,2,512] QK/exp staging, ps 2x[P,512]
(proj/o_proj/transpose/warmup ring), av 2x[P,512] (AV accumulators).
"""

import os

import numpy as np
import ml_dtypes

import concourse.bass as bass
import concourse.mybir as mybir
import concourse.tile as tile
from concourse import bacc
from concourse.bass_utils import run_bass_kernel_spmd
from concourse.masks import make_identity

P = 128
S = 2048
D = 1024
H = 16
DH = 64
HPC = 4          # heads per core
HD = HPC * DH    # 256 model cols per core
KO = D // P      # 8 contraction subtiles for the projections
KT_N = S // P    # 16 key tiles
QT = 512         # query tile
QT_N = S // QT   # 4 query tiles
N_CORES = 8

BF16 = mybir.dt.bfloat16
FP8 = mybir.dt.float8e4
F32 = mybir.dt.float32
DR = mybir.MatmulPerfMode.DoubleRow
EXP = mybir.ActivationFunctionType.Exp

_CACHE = {}
KDBG = os.environ.get("KDBG") == "1"  # debug: dump intermediates to DRAM


def _build_program():
    nc = bacc.Bacc(
        "TRN2",
        target_bir_lowering=False,
        debug=False,
        enable_asserts=False,
        num_devices=N_CORES,
    )
    xT = [nc.dram_tensor(f"xT{t}", [D, S], FP8, kind="ExternalInput").ap()
          for t in "ab"]
    wq = nc.dram_tensor("wq", [D, 2, HD], FP8, kind="ExternalInput").ap()
    wk = nc.dram_tensor("wk", [D, 2, HD], FP8, kind="ExternalInput").ap()
    wv = nc.dram_tensor("wv", [D, 2, HD], FP8, kind="ExternalInput").ap()
    wo = nc.dram_tensor("wo", [HD, D], BF16, kind="ExternalInput").ap()
    bq = nc.dram_tensor("bq", [HD], F32, kind="ExternalInput").ap()
    bk = nc.dram_tensor("bk", [HD], F32, kind="ExternalInput").ap()
    y = nc.dram_tensor("y", [S, D], BF16, kind="ExternalOutput").ap()
    dbg = None
    if KDBG:
        dbg = {
            "kT": nc.dram_tensor("dbg_kT", [P, 2, S], BF16, kind="ExternalOutput").ap(),
            "qT": nc.dram_tensor("dbg_qT", [P, 2, S], BF16, kind="ExternalOutput").ap(),
            "v": nc.dram_tensor("dbg_v", [P, KT_N, HPC, DH + 1], BF16, kind="ExternalOutput").ap(),
            "oN0": nc.dram_tensor("dbg_oN0", [P, 4, HPC, DH], BF16, kind="ExternalOutput").ap(),
            "oT": nc.dram_tensor("dbg_oT", [P, 2, S], BF16, kind="ExternalOutput").ap(),
            "ident": nc.dram_tensor("dbg_ident", [P, P], BF16, kind="ExternalOutput").ap(),
            "exp00": nc.dram_tensor("dbg_exp00", [P, KT_N, QT], BF16, kind="ExternalOutput").ap(),
            "oNe": nc.dram_tensor("dbg_oNe", [P, 4, HPC, DH], BF16, kind="ExternalOutput").ap(),
            "rs0": nc.dram_tensor("dbg_rs0", [P, HPC, 4], F32, kind="ExternalOutput").ap(),
        }

    with tile.TileContext(nc) as tc:
        _body(tc, y, xT, wq, wk, wv, wo, bq, bk, dbg)
    nc.compile()
    return nc


def _body(tc, y, xT, wq, wk, wv, wo, bq, bk, dbg=None):
    nc = tc.nc
    with (
        tc.tile_pool(name="const", bufs=1) as const,
        tc.tile_pool(name="big", bufs=1) as big,
        tc.tile_pool(name="exps", bufs=7) as exps,
        tc.tile_pool(name="onp", bufs=4) as onp,
        tc.tile_pool(name="rsp", bufs=4) as rsp,
        tc.tile_pool(name="yst", bufs=3) as yst,
        tc.tile_pool(name="psQ", bufs=2, space="PSUM") as psQ,
        tc.tile_pool(name="psA", bufs=2, space="PSUM") as psA,
    ):
        # ---- constant / persistent tiles (DMA order = HWDGE issue order;
        # wk + the first xT chunks gate the first QK, so they go first) ----
        # weight a/b halves packed interleaved: 512B rows avoid the
        # sub-512B DMA descriptor penalty, one DMA per weight
        wk_sb = const.tile([P, KO, 2, HD], FP8, tag="wk")
        wq_sb = const.tile([P, KO, 2, HD], FP8, tag="wq")
        xT_sb = [big.tile([P, KO, S], FP8, tag=f"xT{t}", name=f"xT{t}")
                 for t in "ab"]
        wk_r = wk.rearrange("(ko p) t m -> p ko t m", p=P)
        wq_r = wq.rearrange("(ko p) t m -> p ko t m", p=P)
        x_r = [t.rearrange("(ko p) s -> p ko s", p=P) for t in xT]
        for half in range(2):
            ks = slice(4 * half, 4 * half + 4)
            nc.sync.dma_start(wk_sb[:, ks], wk_r[:, ks])
            nc.sync.dma_start(xT_sb[0][:, ks, 0:QT], x_r[0][:, ks, 0:QT])
            nc.sync.dma_start(wq_sb[:, ks], wq_r[:, ks])
            nc.sync.dma_start(xT_sb[1][:, ks, 0:QT], x_r[1][:, ks, 0:QT])

        bk_sb = const.tile([P, HD // P], F32, tag="bk")
        nc.sync.dma_start(bk_sb[:], bk.rearrange("(o p) -> p o", p=P))
        bq_sb = const.tile([P, HD // P], F32, tag="bq")
        nc.sync.dma_start(bq_sb[:], bq.rearrange("(o p) -> p o", p=P))
        for c in range(1, QT_N):
            cs = slice(c * QT, (c + 1) * QT)
            for t in range(2):
                nc.sync.dma_start(xT_sb[t][:, :, cs], x_r[t][:, :, cs])

        wv_sb = const.tile([P, KO, 2, HD], FP8, tag="wv")
        nc.sync.dma_start(wv_sb[:], wv.rearrange("(ko p) t m -> p ko t m", p=P))
        wo_sb = const.tile([P, HD // P, D], BF16, tag="wo")
        nc.sync.dma_start(wo_sb[:], wo.rearrange("(ks p) n -> p ks n", p=P))

        # V with a trailing ones column: col 64 = 1 so the AV matmul emits
        # the sumexp in out column 64.
        v_sb = big.tile([P, KT_N, HPC, DH + 1], BF16, tag="v")
        nc.vector.memset(v_sb[:, :, :, DH : DH + 1], 1.0)

        qT_sb = big.tile([P, HD // P, S], BF16, tag="qT")
        kT_sb = big.tile([P, HD // P, S], BF16, tag="kT")
        oT_sb = big.tile([P, HD // P, S], BF16, tag="oT")

        ident = const.tile([P, P], BF16, tag="ident")
        make_identity(nc, ident[:])

        # ---- PE warmup: dummy matmuls so the p-state ramp is underway ----
        wu = const.tile([P, QT], BF16, tag="wu")
        nc.vector.memset(wu[:], 0.0)
        for i in range(5):
            pwu = psA.tile([P, QT], F32, tag="ps", name="pwu")
            nc.tensor.matmul(
                pwu[:], lhsT=wu[:, 0:P], rhs=wu[:], start=True, stop=True
            )

        # ---- projections: 1-bank [P,512] chunks through the ps ring.
        # fp8 DoubleRow residual: 3 cross terms (xa*wa + xa*wb + xb*wa),
        # each contracting 2 k-tiles per matmul -> 12 matmuls a chunk ----
        TERMS = ((0, 0), (0, 1), (1, 0))

        def proj_mms(out, w_sb, x_pair, mslc, nslc):
            n = 0
            for xt_i, w_i in TERMS:
                for j in range(KO // 2):
                    n += 1
                    nc.tensor.matmul(
                        out,
                        lhsT=w_sb[:, 2 * j : 2 * j + 2, w_i, mslc],
                        rhs=x_pair[xt_i][:, 2 * j : 2 * j + 2, nslc],
                        start=(n == 1),
                        stop=(n == 3 * (KO // 2)),
                        perf_mode=DR,
                    )

        def k_chunk(mt, nt, on_act=False):
            pk = psA.tile([P, QT], F32, tag="ps", name="pk")
            proj_mms(pk[:], wk_sb, xT_sb, slice(mt * P, (mt + 1) * P),
                     slice(nt * QT, (nt + 1) * QT))
            nc.vector.tensor_scalar_add(
                kT_sb[:, mt, nt * QT : (nt + 1) * QT], pk[:],
                bk_sb[:, mt : mt + 1],
            )

        def q_chunk(mt, qt):
            qs = slice(qt * QT, (qt + 1) * QT)
            pq = psA.tile([P, QT], F32, tag="ps", name="pq")
            proj_mms(pq[:], wq_sb, xT_sb, slice(mt * P, (mt + 1) * P), qs)
            nc.vector.tensor_scalar_add(
                qT_sb[:, mt, qs], pq[:], bq_sb[:, mt : mt + 1]
            )

        def v_chunk(st):
            pv = psA.tile([P, QT], F32, tag="ps", name="pv")
            n = 0
            for xt_i, w_i in TERMS:
                for j in range(KO // 2):
                    n += 1
                    nc.tensor.matmul(
                        pv[:, 0:HD],
                        lhsT=xT_sb[xt_i][:, 2 * j : 2 * j + 2, st * P : (st + 1) * P],
                        rhs=wv_sb[:, 2 * j : 2 * j + 2, w_i, :],
                        start=(n == 1),
                        stop=(n == 3 * (KO // 2)),
                        perf_mode=DR,
                    )
            nc.vector.tensor_copy(
                out=v_sb[:, st, :, 0:DH],
                in_=pv[:, 0:HD].rearrange("p (h c) -> p h c", h=HPC),
            )

        # ---- attention ----
        def qk_phase(qt, h, last=False, first=False):
            """QK^T + exp for one (query-tile, head): 8 chunks of [P,2,512]
            (the last group ends with two 1-kt chunks so its final exp --
            which gates the whole tail -- lands sooner)."""
            hb = (h % 2) * DH
            mt = h // 2
            qs = slice(qt * QT, (qt + 1) * QT)
            exp_t = exps.tile([P, KT_N, QT], BF16, tag="exps", name=f"exp{qt}{h}")
            if last:
                widths = [2] * 7 + [1, 1]
            elif first:
                widths = [1, 1] + [2] * 7
            else:
                widths = [2] * 8
            kt = 0
            for w in widths:
                pqk = psQ.tile([P, 2, QT], F32, tag="psq", name="pqk")
                for j in range(w):
                    nc.tensor.matmul(
                        pqk[:, j, :],
                        lhsT=kT_sb[hb : hb + DH, mt, (kt + j) * P : (kt + j + 1) * P],
                        rhs=qT_sb[hb : hb + DH, mt, qs],
                        start=True,
                        stop=True,
                    )
                nc.scalar.activation(
                    exp_t[:, kt : kt + w, :], pqk[:, 0:w, :], EXP,
                    scale=0.125 / 256.0,
                )
                kt += w
            return exp_t

        def transpose_qs(qt, hp, qs, oN):
            # oN [q, 2 heads x 64] -> oT [128 d, q] via a PE transpose (short
            # dependency chain: 53ns matmul + a DVE copy out of PSUM)
            cols = slice(qt * QT + qs * P, qt * QT + (qs + 1) * P)
            tp = psA.tile([P, P], BF16, tag="ps", name="tp")
            nc.tensor.transpose(tp[:], oN[:, qs, 2 * hp : 2 * hp + 2, :], ident[:])
            nc.vector.tensor_copy(out=oT_sb[:, hp, cols], in_=tp[:])

        def o_proj_qs(qt, qs):
            rows = slice(qt * QT + qs * P, qt * QT + (qs + 1) * P)
            for nt in range(D // QT):
                py = psA.tile([P, QT], F32, tag="ps", name="py")
                for ks in range(HD // P):
                    nc.tensor.matmul(
                        py[:],
                        lhsT=oT_sb[:, ks, rows],
                        rhs=wo_sb[:, ks, nt * QT : (nt + 1) * QT],
                        start=(ks == 0),
                        stop=(ks == HD // P - 1),
                    )
                yt = yst.tile([P, QT], BF16, tag="yt", name="yt")
                nc.vector.tensor_copy(out=yt[:], in_=py[:])
                nc.sync.dma_start(y[rows, nt * QT : (nt + 1) * QT], yt[:])

        def o_proj_tail_qs(qt, qs):
            # last-qt variant: QK is done, so the freed psQ ring supplies a
            # 2-bank tile per qs; the PSUM->SBUF copy goes to ACT (idle after
            # the last exp) or DVE, and both nt halves leave in one DMA.
            rows = slice(qt * QT + qs * P, qt * QT + (qs + 1) * P)
            py = psQ.tile([P, 2, QT], F32, tag="psq", name="py")
            yt = yst.tile([P, 2, QT], BF16, tag="yt", name="yt")
            for nt in range(D // QT):
                for ks in range(HD // P):
                    nc.tensor.matmul(
                        py[:, nt, :],
                        lhsT=oT_sb[:, ks, rows],
                        rhs=wo_sb[:, ks, nt * QT : (nt + 1) * QT],
                        start=(ks == 0),
                        stop=(ks == HD // P - 1),
                    )
            if qs in (0, 3):
                nc.scalar.activation(
                    yt[:], py[:], mybir.ActivationFunctionType.Copy
                )
            else:
                nc.vector.tensor_copy(out=yt[:], in_=py[:])
            nc.sync.dma_start(y[rows, :], yt.rearrange("p a b -> p (a b)"))

        def av_phase(qt, h, exp_t, oN, rsum, oproj_inline=False):
            """AV with q on partitions, pipelined per qs128: accumulate
            out[q,65] over kt, then recip/normalize, then (for the heads that
            complete a pair) transpose immediately."""
            # one full PSUM bank per qs accumulation: start=True resets the
            # bank, so concurrent accumulation groups must never share one.
            # The last group borrows the ps ring for qs2/3 so all four
            # accumulations proceed in parallel right after the final exp.
            for qs in range(QT // P):
                tag = "ps" if (oproj_inline and qs >= 2) else "av"
                av = psA.tile([P, QT], F32, tag=tag, name="av")
                for kt in range(KT_N):
                    nc.tensor.matmul(
                        av[:, 0 : DH + 1],
                        lhsT=exp_t[:, kt, qs * P : (qs + 1) * P],
                        rhs=v_sb[:, kt, h, :],
                        start=(kt == 0),
                        stop=(kt == KT_N - 1),
                    )
                nc.vector.reciprocal(
                    rsum[:, h, qs : qs + 1], av[:, DH : DH + 1]
                )
                nc.vector.tensor_scalar_mul(
                    oN[:, qs, h, :], av[:, 0:DH], rsum[:, h, qs : qs + 1]
                )
            # transposes emitted after the whole AV block: their matmuls wait
            # on the DVE norms, and emitting them mid-block head-blocks the
            # in-order PE queue behind that wait
            if h == 1:
                for qs in range(QT // P):
                    transpose_qs(qt, 0, qs, oN)
            elif h == 3:
                if oproj_inline:
                    # tail: interleave per qs so copies/DMAs start early
                    for qs in range(QT // P):
                        transpose_qs(qt, 1, qs, oN)
                        o_proj_tail_qs(qt, qs)
                else:
                    for qs in range(QT // P):
                        transpose_qs(qt, 1, qs, oN)

        # ---- emission schedule: two head-pair passes (all qt with heads
        # 0/1, then heads 2/3) so the mt=1 K/Q projections move out of the
        # congested early phase; per-group lookahead keeps ACT fed. ----
        groups = [(qt, h) for hp in range(2) for qt in range(QT_N)
                  for h in (2 * hp, 2 * hp + 1)]
        # every tensor must be fully emitted BEFORE its consumer (Tile deps
        # follow program order; a read emitted ahead of its producer races)
        k_chunk(0, 0, on_act=True)
        q_chunk(0, 0)
        for nt in range(1, QT_N):
            k_chunk(0, nt)
        e = {groups[0]: qk_phase(*groups[0], first=True)}
        if dbg is not None:
            nc.sync.dma_start(dbg["exp00"], e[groups[0]][:])
        e[groups[1]] = qk_phase(*groups[1])

        oN, rsum = {}, {}

        def ensure_tiles(qt):
            if qt not in oN:
                oN[qt] = onp.tile(
                    [P, QT // P, HPC, DH], BF16, tag="oN", name=f"oN{qt}"
                )
                rsum[qt] = rsp.tile(
                    [P, HPC, QT // P], F32, tag="rs", name=f"rs{qt}"
                )

        ensure_tiles(0)

        def run_av(j):
            qtj, hj = groups[j]
            av_phase(
                qtj, hj, e.pop(groups[j]), oN[qtj], rsum[qtj],
                oproj_inline=(j == len(groups) - 1),
            )
            if hj == 3 and qtj != QT_N - 1:
                for qs in range(QT // P):
                    o_proj_qs(qtj, qs)
            if dbg is not None and j == 1:
                nc.sync.dma_start(dbg["oNe"], oN[0][:])

        # AV consumption lags QK emission by 2 groups: the exp pool absorbs
        # the slack, and all projection chunks land before their consumers.
        for i, g in enumerate(groups):
            qt, h = g
            if i < 3:
                for st in range(6 * i, min(6 * i + 6, KT_N)):
                    v_chunk(st)
            if h == 0 and i < 8 and qt + 1 < QT_N:
                q_chunk(0, qt + 1)          # q mt0 for the next pass-A qt
            if 3 <= i < 7:
                k_chunk(1, i - 3)           # K mt1, needed from group 8 on
            if i == 5:
                q_chunk(1, 0)               # q mt1 for (0,2)/(0,3)
            if i == 7:
                q_chunk(1, 1)
            if h == 3 and 8 <= i < 13:
                q_chunk(1, qt + 2)          # q mt1 for later pass-B qts
            if i + 2 < len(groups):
                g2 = groups[i + 2]
                ensure_tiles(g2[0])
                e[g2] = qk_phase(*g2, last=(i + 2 == len(groups) - 1))
            if i >= 3:
                run_av(i - 3)
        for j in range(len(groups) - 3, len(groups)):
            run_av(j)
        if dbg is not None:
            nc.sync.dma_start(dbg["rs0"], rsum[0][:])
            nc.sync.dma_start(dbg["kT"], kT_sb[:])
            nc.sync.dma_start(dbg["qT"], qT_sb[:])
            nc.sync.dma_start(dbg["v"], v_sb[:])
            nc.sync.dma_start(dbg["oN0"], oN[0][:])
            nc.sync.dma_start(dbg["oT"], oT_sb[:])
            nc.sync.dma_start(dbg["ident"], ident[:])


def _fp8_split(a):
    f8 = mybir.dt.np(mybir.dt.float8e4)
    hi = np.ascontiguousarray(a).astype(f8)
    lo = np.ascontiguousarray(a - hi.astype(np.float32)).astype(f8)
    return hi, lo


def _prep_inputs(x, Wq, bq, Wk, bk, Wv, bv, Wo, bo):
    bf = ml_dtypes.bfloat16
    x = np.asarray(x, np.float32)
    in_maps = []
    for c in range(N_CORES):
        b, hq = c // 4, c % 4
        cs = slice(hq * HD, (hq + 1) * HD)
        xa, xb = _fp8_split(x[b].T)
        # weights x16 so fp8 residuals stay out of the subnormal range; the
        # scale cancels: S x256 is absorbed by the exp scale, V x16 by Wo/16
        wqa, wqb = _fp8_split(16.0 * np.asarray(Wq, np.float32)[:, cs])
        wka, wkb = _fp8_split(16.0 * np.asarray(Wk, np.float32)[:, cs])
        wva, wvb = _fp8_split(16.0 * np.asarray(Wv, np.float32)[:, cs])
        in_maps.append(
            {
                "xTa": xa, "xTb": xb,
                "wq": np.ascontiguousarray(np.stack([wqa, wqb], axis=1)),
                "wk": np.ascontiguousarray(np.stack([wka, wkb], axis=1)),
                "wv": np.ascontiguousarray(np.stack([wva, wvb], axis=1)),
                "wo": np.ascontiguousarray(
                    np.asarray(Wo, np.float32)[cs, :] / 16.0
                ).astype(bf),
                "bq": np.ascontiguousarray(16.0 * np.asarray(bq, np.float32)[cs]),
                "bk": np.ascontiguousarray(16.0 * np.asarray(bk, np.float32)[cs]),
            }
        )
    return in_maps


def get_program():
    if "nc" not in _CACHE:
        _CACHE["nc"] = _build_program()
    return _CACHE["nc"]


def run(inputs, **kw):
    nc = get_program()
    in_maps = _prep_inputs(**inputs)
    res = run_bass_kernel_spmd(nc, in_maps, core_ids=list(range(N_CORES)), **kw)
    # final bias: bo + bv @ Wo (bv folds out of attention since softmax rows
    # sum to 1), computed in fp32 on host
    bias = np.asarray(inputs["bo"], np.float32) + np.asarray(
        inputs["bv"], np.float32
    ) @ np.asarray(inputs["Wo"], np.float32)
    out = np.empty((2, S, D), np.float32)
    for b in range(2):
        acc = res.results[4 * b]["y"].astype(np.float32).copy()
        for i in range(1, 4):
            acc += res.results[4 * b + i]["y"]
        out[b] = acc + bias
    return out, res


def kernel(**inputs):
    out, _ = run(inputs)
    return out



# revision 23
# speedup vs baseline: 1.0870x; 1.0870x over previous
"""Trainium2 Bass kernel for CustomMultiHeadAttention.

Problem: x[2,2048,1024], 16 heads, Dh=64. y = MHA(x) with Q/K/V/O projections.

Sharding (8 cores, no collectives):
  core c -> batch b = c//4, head-quarter hq = c%4 (4 heads, 256 model cols).
  Each core computes Q,K,V projections for its 4 heads over the FULL batch
  sequence, attention for those heads, and a PARTIAL o_proj (its 256 rows of
  Wo). The host sums the 4 bf16 partials per batch in fp32 and adds
  bo + bv@Wo (bv folds out of attention since softmax rows sum to 1).

Projections (fp8 DoubleRow residual):
  x ships as fp8e4m3 (hi, lo) pairs scaled by 0.3003 and Wq/Wk/Wv x16, so
  Q/K land in PSUM already at the exp2-friendly scale g = sqrt(16*log2 e)
  = 4.8045: S_psum = g^2 qk = 128*log2(e)*(qk/8) -- the bf16 exponent-field
  value of exp(qk/8) directly. Each projection sums the 3 fp8 cross terms
  with DoubleRow matmuls (2 k-tiles per instruction, 0.5 cycles/row).

QK at fp8-DoubleRow rate (the 4-cross-term packing):
  K and Q are re-quantized on-chip to fp8 (hi, lo) residual pairs by DVE.
  K tile [128, 2, S]: partitions 0-63 hold head dims 0-63 as interleaved
  (hi, lo) pairs; partitions 64-127 duplicate them (SBUF->SBUF DMA).
  Q tile [128, S]: partitions 0-63 = Q_hi, 64-127 = Q_lo of the same dims;
  the matmul rhs broadcasts the pair axis with a 0-stride AP. One DoubleRow
  matmul then contracts all 4 cross terms (Khi+Klo)(Qhi+Qlo) = full-precision
  K.Q at 0.5 cycles/row -- 2x the bf16 rate, fp8-pair (~bf16) accuracy.

Softmax exp split across ACT and DVE:
  ACT chunks: activation(Exp, scale=ln2/128) on [P,2,512] PSUM chunks.
  DVE chunks: custom 8-stage DVE op EXP2C_ANT computes the bf16 BITS of
  2^(y/128) as int16: y'=y-64; k=((y'+M)-M) with M=1.5*2^30 floors y to a
  multiple of 128; i = y' + 0.0027*(y'-k)^2 + D. Round-to-nearest int16
  convert + bf16 bitcast yield exp with ~0.25% RMS error whose constant
  factor cancels in the softmax normalization. This offloads a tunable
  fraction of the exp stream from the saturated ACT engine onto DVE.

Attention dataflow (as baseline):
  S^T per (head, kt, qt512) in PSUM; P^T -> exps pool bf16;
  AV with q on partitions + ones column for sumexp; DVE recip+scale;
  PE transpose; o_proj bf16 with partial-Wo; host reduce.

PSUM (8 banks): psQ 2x[P,2,512] QK/exp staging, ps 2x[P,512]
(proj/o_proj/transpose/warmup ring), av 2x[P,512] (AV accumulators).
"""

import os

import numpy as np
import ml_dtypes

import concourse.bass as bass
import concourse.mybir as mybir
import concourse.tile as tile
from concourse import bacc
from concourse.bass_utils import run_bass_kernel_spmd
from concourse.masks import make_identity

# ---- custom DVE op: bf16-bitcast exp2 with quadratic mantissa correction ----
from concourse import dve_ops as _DO
from concourse.dve_spec import Spec, Src0, Src1, C0, C1, C2, lower as _dve_lower, sq as _dve_sq
from concourse.dve_uop import DveOpSpec as _DveOpSpec

EXP2_MAGIC = 1.5 * (2.0 ** 30)
EXP2_C2 = 0.0027
EXP2_D = 16309.036


def _exp2_ref(in0, in1, s0, s1, imm2):
    y1 = (in0.astype(np.float32) - np.float32(s0)).astype(np.float32)
    z = (y1 + np.float32(s1)).astype(np.float32)
    k = (z - np.float32(s1)).astype(np.float32)
    t = (y1 - k).astype(np.float32)
    return ((t * t).astype(np.float32) * np.float32(imm2) + y1).astype(
        np.float32
    ) + in1


def _register_exp2_op():
    name = "EXP2C_ANT"
    for o in _DO.OPS:
        if o.name == name:
            return o
    _y1 = Src0 - C0
    _z = _y1 + C1
    _k = _z - C1
    _t = _y1 - _k
    spec = Spec(body=(_dve_sq(_t) * C2 + _y1) + Src1, reference=_exp2_ref)
    row = _DO._CUSTOM_DVE_ROW_BASE + len(_DO.OPS)
    assert row < 0x20
    shas = {}
    for ver in ("v3", "v4"):
        s = _DveOpSpec(name=name, opcode=row, uops=_dve_lower(spec, ver=ver), rd1_en=True)
        shas[ver] = s.sha(ver)
    op = _DO.DveOp(name, spec, subdim=False, uops_sha=shas)
    _DO.OPS.append(op)
    _DO._SUB_OPCODE_FOR_NAME[name] = row
    _DO.CUSTOM_DVE_SPECS[name] = spec
    return op


EXP2_OP = _register_exp2_op()

P = 128
S = 2048
D = 1024
H = 16
DH = 64
HPC = 4          # heads per core
HD = HPC * DH    # 256 model cols per core
KO = D // P      # 8 contraction subtiles for the projections
KT_N = S // P    # 16 key tiles
QT = 512         # query tile
QT_N = S // QT   # 4 query tiles
N_CORES = 8

BF16 = mybir.dt.bfloat16
FP8 = mybir.dt.float8e4
F32 = mybir.dt.float32
U16 = mybir.dt.uint16
DR = mybir.MatmulPerfMode.DoubleRow
EXP = mybir.ActivationFunctionType.Exp
ADD = mybir.AluOpType.add
SUB = mybir.AluOpType.subtract

# scale split: x staged at XS, Wq/Wk at 16 -> S_psum = 128*log2(e)*S_true/8
G2 = 128.0 * np.log2(np.e) * 0.125          # 23.0831...
XS = np.sqrt(G2) / 16.0                     # 0.300281...
ACT_EXP_SCALE = float(np.log(2.0) / 128.0)  # exp(S_psum * this) = exp(qk/8)

# which psQ chunks of each qk phase go to the DVE exp2 path (of ~8)
DVE_EXP_IDX = frozenset((2, 6))

_CACHE = {}
KDBG = os.environ.get("KDBG") == "1"  # debug: dump intermediates to DRAM


def _build_program():
    nc = bacc.Bacc(
        "TRN2",
        target_bir_lowering=False,
        debug=False,
        enable_asserts=False,
        num_devices=N_CORES,
    )
    xT = [nc.dram_tensor(f"xT{t}", [D, S], FP8, kind="ExternalInput").ap()
          for t in "ab"]
    wq = nc.dram_tensor("wq", [D, 2, HD], FP8, kind="ExternalInput").ap()
    wk = nc.dram_tensor("wk", [D, 2, HD], FP8, kind="ExternalInput").ap()
    wv = nc.dram_tensor("wv", [D, 2, HD], FP8, kind="ExternalInput").ap()
    wo = nc.dram_tensor("wo", [HD, D], BF16, kind="ExternalInput").ap()
    bq = nc.dram_tensor("bq", [HD], F32, kind="ExternalInput").ap()
    bk = nc.dram_tensor("bk", [HD], F32, kind="ExternalInput").ap()
    y = nc.dram_tensor("y", [S, D], BF16, kind="ExternalOutput").ap()
    dbg = None
    if KDBG:
        dbg = {
            "kstg": nc.dram_tensor("dbg_kstg", [P, 2, S], FP8, kind="ExternalOutput").ap(),
            "qstg": nc.dram_tensor("dbg_qstg", [P, 2, S], FP8, kind="ExternalOutput").ap(),
            "ktile": nc.dram_tensor("dbg_ktile", [P, HPC, 2, S], FP8, kind="ExternalOutput").ap(),
            "qtile": nc.dram_tensor("dbg_qtile", [P, HPC, S], FP8, kind="ExternalOutput").ap(),
            "v": nc.dram_tensor("dbg_v", [P, KT_N, HPC, DH + 1], BF16, kind="ExternalOutput").ap(),
            "exp00": nc.dram_tensor("dbg_exp00", [P, KT_N, QT], BF16, kind="ExternalOutput").ap(),
            "oN0": nc.dram_tensor("dbg_oN0", [P, 4, HPC, DH], BF16, kind="ExternalOutput").ap(),
        }

    with tile.TileContext(nc) as tc:
        _body(tc, y, xT, wq, wk, wv, wo, bq, bk, dbg)
    nc.compile()
    return nc


def _body(tc, y, xT, wq, wk, wv, wo, bq, bk, dbg=None):
    nc = tc.nc
    with (
        tc.tile_pool(name="const", bufs=1) as const,
        tc.tile_pool(name="big", bufs=1) as big,
        tc.tile_pool(name="exps", bufs=5) as exps,
        tc.tile_pool(name="onp", bufs=4) as onp,
        tc.tile_pool(name="rsp", bufs=4) as rsp,
        tc.tile_pool(name="yst", bufs=3) as yst,
        tc.tile_pool(name="psQ", bufs=2, space="PSUM") as psQ,
        tc.tile_pool(name="psA", bufs=2, space="PSUM") as psA,
    ):
        # ---- constant / persistent tiles (DMA order = HWDGE issue order;
        # wk + the first xT chunks gate the first QK, so they go first) ----
        wk_sb = const.tile([P, KO, 2, HD], FP8, tag="wk")
        wq_sb = const.tile([P, KO, 2, HD], FP8, tag="wq")
        xT_sb = [big.tile([P, KO, S], FP8, tag=f"xT{t}", name=f"xT{t}")
                 for t in "ab"]
        wk_r = wk.rearrange("(ko p) t m -> p ko t m", p=P)
        wq_r = wq.rearrange("(ko p) t m -> p ko t m", p=P)
        x_r = [t.rearrange("(ko p) s -> p ko s", p=P) for t in xT]
        for half in range(2):
            ks = slice(4 * half, 4 * half + 4)
            nc.sync.dma_start(wk_sb[:, ks], wk_r[:, ks])
            nc.sync.dma_start(xT_sb[0][:, ks, 0:QT], x_r[0][:, ks, 0:QT])
            nc.sync.dma_start(wq_sb[:, ks], wq_r[:, ks])
            nc.sync.dma_start(xT_sb[1][:, ks, 0:QT], x_r[1][:, ks, 0:QT])

        bk_sb = const.tile([P, HD // P], F32, tag="bk")
        nc.sync.dma_start(bk_sb[:], bk.rearrange("(o p) -> p o", p=P))
        bq_sb = const.tile([P, HD // P], F32, tag="bq")
        nc.sync.dma_start(bq_sb[:], bq.rearrange("(o p) -> p o", p=P))
        for c in range(1, QT_N):
            cs = slice(c * QT, (c + 1) * QT)
            for t in range(2):
                nc.sync.dma_start(xT_sb[t][:, :, cs], x_r[t][:, :, cs])

        wv_sb = const.tile([P, KO, 2, HD], FP8, tag="wv")
        nc.sync.dma_start(wv_sb[:], wv.rearrange("(ko p) t m -> p ko t m", p=P))
        wo_sb = const.tile([P, HD // P, D], BF16, tag="wo")
        nc.sync.dma_start(wo_sb[:], wo.rearrange("(ks p) n -> p ks n", p=P))

        # V with a trailing ones column: col 64 = 1 so the AV matmul emits
        # the sumexp in out column 64.
        v_sb = big.tile([P, KT_N, HPC, DH + 1], BF16, tag="v")
        nc.vector.memset(v_sb[:, :, :, DH : DH + 1], 1.0)

        # K/Q fp8 residual-pair stores.
        # kstg/qstg: undup'd staging [dims, (hi,lo), S] per mt, written by DVE
        # from the projection PSUM. ktile[h]: partitions 0-63 = head dims 0-63
        # as (hi,lo) pairs, 64-127 duplicate. qtile[h]: 0-63 = hi, 64-127 = lo.
        kstg = [big.tile([P, 2, S], FP8, tag=f"kstg{mt}", name=f"kstg{mt}")
                for mt in range(2)]
        qstg = [big.tile([P, 2, S], FP8, tag=f"qstg{mt}", name=f"qstg{mt}")
                for mt in range(2)]
        ktile = [big.tile([P, 2, S], FP8, tag=f"ktile{h}", name=f"ktile{h}")
                 for h in range(HPC)]
        qtile = [big.tile([P, S], FP8, tag=f"qtile{h}", name=f"qtile{h}")
                 for h in range(HPC)]

        oT_sb = big.tile([P, HD // P, S], BF16, tag="oT")

        # D-constant tile for the custom exp2 op's final add
        dtile = const.tile([P, 2, QT], F32, tag="dtile")
        nc.vector.memset(dtile[:], EXP2_D)

        ident = const.tile([P, P], BF16, tag="ident")
        make_identity(nc, ident[:])

        # ---- PE warmup: dummy matmuls so the p-state ramp is underway ----
        wu = const.tile([P, QT], BF16, tag="wu")
        nc.vector.memset(wu[:], 0.0)
        for i in range(5):
            pwu = psA.tile([P, QT], F32, tag="ps", name="pwu")
            nc.tensor.matmul(
                pwu[:], lhsT=wu[:, 0:P], rhs=wu[:], start=True, stop=True
            )

        # ---- projections: 1-bank [P,512] chunks through the ps ring.
        # fp8 DoubleRow residual: 3 cross terms (xa*wa + xa*wb + xb*wa),
        # each contracting 2 k-tiles per matmul -> 12 matmuls a chunk ----
        TERMS = ((0, 0), (0, 1), (1, 0))

        def proj_mms(out, w_sb, x_pair, mslc, nslc):
            n = 0
            for xt_i, w_i in TERMS:
                for j in range(KO // 2):
                    n += 1
                    nc.tensor.matmul(
                        out,
                        lhsT=w_sb[:, 2 * j : 2 * j + 2, w_i, mslc],
                        rhs=x_pair[xt_i][:, 2 * j : 2 * j + 2, nslc],
                        start=(n == 1),
                        stop=(n == 3 * (KO // 2)),
                        perf_mode=DR,
                    )

        def stage_pair(stg, psum, bias_ap, cols):
            # hi = fp8(psum + bias); lo = fp8((psum + bias) - hi)
            nc.vector.tensor_scalar_add(stg[:, 0, cols], psum, bias_ap)
            nc.vector.scalar_tensor_tensor(
                out=stg[:, 1, cols], in0=psum, scalar=bias_ap,
                in1=stg[:, 0, cols], op0=ADD, op1=SUB,
            )

        def k_chunk(mt, nt):
            pk = psA.tile([P, QT], F32, tag="ps", name="pk")
            proj_mms(pk[:], wk_sb, xT_sb, slice(mt * P, (mt + 1) * P),
                     slice(nt * QT, (nt + 1) * QT))
            stage_pair(kstg[mt], pk[:], bk_sb[:, mt : mt + 1],
                       slice(nt * QT, (nt + 1) * QT))

        def q_chunk(mt, qt):
            qs = slice(qt * QT, (qt + 1) * QT)
            pq = psA.tile([P, QT], F32, tag="ps", name="pq")
            proj_mms(pq[:], wq_sb, xT_sb, slice(mt * P, (mt + 1) * P), qs)
            stage_pair(qstg[mt], pq[:], bq_sb[:, mt : mt + 1], qs)

        def dup_k(h, part=None):
            # ktile[h][0:64] and [64:128] <- kstg[mt][hb:hb+64]
            mt, hb = h // 2, (h % 2) * DH
            cols = slice(None) if part is None else slice(part * QT, (part + 1) * QT)
            src = kstg[mt][hb : hb + DH, :, cols]
            nc.sync.dma_start(ktile[h][0:DH, :, cols], src)
            nc.sync.dma_start(ktile[h][DH:P, :, cols], src)

        def dup_q(h, qt):
            # qtile[h][0:64] <- hi, [64:128] <- lo
            mt, hb = h // 2, (h % 2) * DH
            cols = slice(qt * QT, (qt + 1) * QT)
            nc.sync.dma_start(qtile[h][0:DH, cols], qstg[mt][hb : hb + DH, 0, cols])
            nc.sync.dma_start(qtile[h][DH:P, cols], qstg[mt][hb : hb + DH, 1, cols])

        def v_chunk(st):
            pv = psA.tile([P, QT], F32, tag="ps", name="pv")
            n = 0
            for xt_i, w_i in TERMS:
                for j in range(KO // 2):
                    n += 1
                    nc.tensor.matmul(
                        pv[:, 0:HD],
                        lhsT=xT_sb[xt_i][:, 2 * j : 2 * j + 2, st * P : (st + 1) * P],
                        rhs=wv_sb[:, 2 * j : 2 * j + 2, w_i, :],
                        start=(n == 1),
                        stop=(n == 3 * (KO // 2)),
                        perf_mode=DR,
                    )
            nc.vector.tensor_copy(
                out=v_sb[:, st, :, 0:DH],
                in_=pv[:, 0:HD].rearrange("p (h c) -> p h c", h=HPC),
            )

        # ---- attention ----
        def qk_chunk_act(exp_t, h, rhs, kt, w):
            """One ACT psQ chunk of a QK phase: w DoubleRow matmuls + exp.
            The psQ ring carries ONLY ACT chunks, so the exp(c)->QK(c+2)
            chain latency always hides behind exp(c+1) and ACT is gapless."""
            pqk = psQ.tile([P, 2, QT], F32, tag="psq", name="pqk")
            for j in range(w):
                nc.tensor.matmul(
                    pqk[:, j, :],
                    lhsT=ktile[h][:, :, (kt + j) * P : (kt + j + 1) * P],
                    rhs=rhs,
                    start=True,
                    stop=True,
                    perf_mode=DR,
                )
            nc.scalar.activation(
                exp_t[:, kt : kt + w, :], pqk[:, 0:w, :], EXP,
                scale=ACT_EXP_SCALE,
            )

        def qk_chunk_dve(exp_t, h, rhs, kt):
            """One DVE single-kt chunk, staged through the av ring (1 bank)
            so it never blocks the ACT psQ ring."""
            pq1 = psA.tile([P, QT], F32, tag="av", name="dq")
            nc.tensor.matmul(
                pq1[:],
                lhsT=ktile[h][:, :, kt * P : (kt + 1) * P],
                rhs=rhs,
                start=True,
                stop=True,
                perf_mode=DR,
            )
            nc.vector._custom_dve(
                EXP2_OP,
                out=exp_t[:, kt, :].bitcast(U16),
                in0=pq1[:],
                in1=dtile[:, 0, :],
                s0=64.0, s1=EXP2_MAGIC, imm2=EXP2_C2,
            )

        # per-phase chunk patterns: (width, is_dve). DVE singles carry 4 of
        # the 16 kt (f=0.25); the last phases shift more onto DVE so the
        # tail exp drains fast.
        PAT_STD = [(2, 0), (2, 0), (1, 1), (2, 0), (1, 1),
                   (2, 0), (2, 0), (1, 1), (2, 0), (1, 1)]
        PAT_FIRST = [(1, 0), (1, 0), (2, 0), (1, 1), (2, 0), (1, 1),
                     (2, 0), (2, 0), (1, 1), (2, 0), (1, 1)]
        PAT_LAST = [(2, 0), (1, 1), (2, 0), (1, 1), (2, 0), (1, 1),
                    (2, 0), (1, 1), (2, 0), (1, 1), (1, 0)]

        def qk_units(qt, h, last=False, first=False):
            """Closures for one (query-tile, head) QK+exp phase, one per
            chunk, so other PE work can be woven between chunks (the in-order
            PE queue would otherwise head-block on the psum rings)."""
            qs = slice(qt * QT, (qt + 1) * QT)
            exp_t = exps.tile([P, KT_N, QT], BF16, tag="exps", name=f"exp{qt}{h}")
            rhs = qtile[h][:, qs].unsqueeze(1).broadcast_to([P, 2, QT])
            pat = PAT_LAST if last else (PAT_FIRST if first else PAT_STD)
            units = []
            kt = 0
            for w, dve in pat:
                if dve:
                    units.append(lambda kt=kt: qk_chunk_dve(exp_t, h, rhs, kt))
                else:
                    units.append(lambda kt=kt, w=w: qk_chunk_act(exp_t, h, rhs, kt, w))
                kt += w
            return exp_t, units

        def transpose_qs(qt, hp, qs, oN, on_act=False):
            # oN [q, 2 heads x 64] -> oT [128 d, q] via a PE transpose
            cols = slice(qt * QT + qs * P, qt * QT + (qs + 1) * P)
            tp = psA.tile([P, P], BF16, tag="ps", name="tp")
            nc.tensor.transpose(tp[:], oN[:, qs, 2 * hp : 2 * hp + 2, :], ident[:])
            if on_act:
                nc.scalar.activation(oT_sb[:, hp, cols], tp[:],
                                     mybir.ActivationFunctionType.Copy)
            else:
                nc.vector.tensor_copy(out=oT_sb[:, hp, cols], in_=tp[:])

        def o_proj_qs(qt, qs, copy_act=False):
            rows = slice(qt * QT + qs * P, qt * QT + (qs + 1) * P)
            for nt in range(D // QT):
                py = psA.tile([P, QT], F32, tag="ps", name="py")
                for ks in range(HD // P):
                    nc.tensor.matmul(
                        py[:],
                        lhsT=oT_sb[:, ks, rows],
                        rhs=wo_sb[:, ks, nt * QT : (nt + 1) * QT],
                        start=(ks == 0),
                        stop=(ks == HD // P - 1),
                    )
                yt = yst.tile([P, QT], BF16, tag="yt", name="yt")
                if copy_act:
                    nc.scalar.activation(yt[:], py[:],
                                         mybir.ActivationFunctionType.Copy)
                else:
                    nc.vector.tensor_copy(out=yt[:], in_=py[:])
                nc.sync.dma_start(y[rows, nt * QT : (nt + 1) * QT], yt[:])

        def o_proj_tail_qs(qt, qs):
            # last-qt variant: QK is done, so the freed psQ ring supplies a
            # 2-bank tile per qs; the PSUM->SBUF copy goes to ACT (idle after
            # the last exp) or DVE, and both nt halves leave in one DMA.
            rows = slice(qt * QT + qs * P, qt * QT + (qs + 1) * P)
            py = psQ.tile([P, 2, QT], F32, tag="psq", name="py")
            yt = yst.tile([P, 2, QT], BF16, tag="yt", name="yt")
            for nt in range(D // QT):
                for ks in range(HD // P):
                    nc.tensor.matmul(
                        py[:, nt, :],
                        lhsT=oT_sb[:, ks, rows],
                        rhs=wo_sb[:, ks, nt * QT : (nt + 1) * QT],
                        start=(ks == 0),
                        stop=(ks == HD // P - 1),
                    )
            nc.scalar.activation(
                yt[:, 0, :], py[:, 0, :], mybir.ActivationFunctionType.Copy
            )
            nc.vector.tensor_copy(out=yt[:, 1, :], in_=py[:, 1, :])
            nc.sync.dma_start(y[rows, :], yt.rearrange("p a b -> p (a b)"))

        def av_qs(qt, h, exp_t, oN, rsum, qs, oproj_inline=False):
            """One qs128 slice of AV: accumulate out[q,65] over kt into a full
            PSUM bank (start=True resets the bank), then recip/normalize."""
            tag = "ps" if (oproj_inline and qs >= 2) else "av"
            av = psA.tile([P, QT], F32, tag=tag, name="av")
            for kt in range(KT_N):
                nc.tensor.matmul(
                    av[:, 0 : DH + 1],
                    lhsT=exp_t[:, kt, qs * P : (qs + 1) * P],
                    rhs=v_sb[:, kt, h, :],
                    start=(kt == 0),
                    stop=(kt == KT_N - 1),
                )
            nc.vector.reciprocal(
                rsum[:, h, qs : qs + 1], av[:, DH : DH + 1]
            )
            nc.vector.tensor_scalar_mul(
                oN[:, qs, h, :], av[:, 0:DH], rsum[:, h, qs : qs + 1]
            )

        def av_units(qt, h, exp_t, oN, rsum, oproj_inline=False):
            """Closures for one AV group: 4 av_qs, then (for pair-completing
            heads) transposes, then o_proj. Transposes come after the whole
            AV block: their matmuls wait on the DVE norms, and emitting them
            mid-block head-blocks the in-order PE queue behind that wait."""
            units = [
                lambda qs=qs: av_qs(qt, h, exp_t, oN, rsum, qs, oproj_inline)
                for qs in range(QT // P)
            ]
            if h == 1:
                units += [lambda qs=qs: transpose_qs(qt, 0, qs, oN)
                          for qs in range(QT // P)]
            elif h == 3:
                late = qt >= 2
                trs = [lambda qs=qs, a=late: transpose_qs(qt, 1, qs, oN, a)
                       for qs in range(QT // P)]
                if oproj_inline:
                    # tail: interleave av/transpose/o_proj per qs so the
                    # final chain pipelines instead of running in strata
                    opj = [lambda qs=qs: o_proj_tail_qs(qt, qs)
                           for qs in range(QT // P)]
                    units = [units[0], units[1], units[2], trs[0], units[3],
                             trs[1], opj[0], trs[2], opj[1], trs[3],
                             opj[2], opj[3]]
                elif qt != QT_N - 1:
                    units += trs + [lambda qs=qs, a=late: o_proj_qs(qt, qs, a)
                                    for qs in range(QT // P)]
                    trs = None
                if trs is not None and not oproj_inline:
                    units += trs
            return units

        # ---- emission schedule: two head-pair passes (all qt with heads
        # 0/1, then heads 2/3). qk phase i is emitted AT iter i, chunk-woven
        # with AV of phase i-3 and the projection/dup work scheduled for the
        # iter, so the in-order PE queue never head-blocks on the psQ ring
        # (QK at fp8-DR rate is ~5x faster than the ACT/DVE exp drain). ----
        groups = [(qt, h) for hp in range(2) for qt in range(QT_N)
                  for h in (2 * hp, 2 * hp + 1)]
        AV_LAG = 2

        oN, rsum = {}, {}

        def ensure_tiles(qt):
            if qt not in oN:
                oN[qt] = onp.tile(
                    [P, QT // P, HPC, DH], BF16, tag="oN", name=f"oN{qt}"
                )
                rsum[qt] = rsp.tile(
                    [P, HPC, QT // P], F32, tag="rs", name=f"rs{qt}"
                )

        e = {}

        def av_units_for(j):
            qtj, hj = groups[j]
            return av_units(
                qtj, hj, e.pop(groups[j]), oN[qtj], rsum[qtj],
                oproj_inline=(j == len(groups) - 1),
            )

        backlog = []  # ordered (key, fn, pe_ns) filler units; drained lazily

        def drain_until(key):
            # force-emit backlog items up to and including `key` (producer
            # guarantees: a consumer may only be emitted after its producer)
            if not any(k == key for k, _, _ in backlog):
                return
            while backlog:
                k, fn, _ = backlog.pop(0)
                fn()
                if k == key:
                    return

        def weave(qk, budgets):
            # prime both psQ slots, then pace fillers by estimated PE time.
            # budgets[i] is the filler time allowed before qk chunk i: the
            # exp(c)->QK(c+2) chain hides behind exp(c+1) on the other psQ
            # slot, EXCEPT right after a DVE chunk (ACT is then waiting
            # immediately), so those positions get zero budget.
            qk[0](); qk[1]()
            for u, b in zip(qk[2:], budgets[2:]):
                cover = 0.0
                while cover < b and backlog:
                    _, fn, ns = backlog.pop(0)
                    fn()
                    cover += ns
                u()

        # --- startup: x chunks land one by one (~3.2us apart); emit every
        # projection chunk (both mt, + fp8 staging + dup DMAs) as its x chunk
        # arrives, with the part-gated first QK phase woven in so ACT starts
        # early. Every unit here is emitted strictly after its producers
        # (Tile deps follow program order). ---
        # Keep pre-phase-1 PE work UNDER the ~13us x-DMA gate: only K mt0,
        # q(0,0), and the part-gated first phase. Everything else (K mt1, V,
        # q mt1) defers into the backlog.
        ensure_tiles(0)
        k_chunk(0, 0); dup_k(0, 0); dup_k(1, 0)
        q_chunk(0, 0); dup_q(0, 0); dup_q(1, 0)
        e[groups[0]], qk0 = qk_units(0, 0, first=True)
        qk0[0](); qk0[1](); qk0[2]()                  # kt 0-3 (part 0)
        k_chunk(0, 1); dup_k(0, 1); dup_k(1, 1)
        qk0[3](); qk0[4](); qk0[5]()                  # kt 4-7 (part 1)
        k_chunk(0, 2); dup_k(0, 2); dup_k(1, 2)
        qk0[6](); qk0[7]()                            # kt 8-11 (part 2)
        k_chunk(0, 3); dup_k(0, 3); dup_k(1, 3)
        qk0[8](); qk0[9](); qk0[10]()                 # kt 12-15 (part 3)
        if dbg is not None:
            nc.sync.dma_start(dbg["exp00"], e[groups[0]][:])

        # --- iters 1..15: qk(i) woven with the filler backlog (av(i-3),
        # transposes, o_proj, remaining projections, v chunks) ---
        def av_due(i):
            # deferred AV ramp: the first groups wait until the x-DMA-gated
            # startup projections/v-chunks have drained, then settle to lag
            # 3, and the last iters tighten to lag 2 so the tail is short
            return {13: [10, 11], 14: [12, 13], 15: [14]}.get(
                i, [i - 3] if 3 <= i <= 12 else [])

        for i in range(1, len(groups)):
            qt, h = groups[i]
            if i == 1:
                q_chunk(0, 1); dup_q(0, 1); dup_q(1, 1)
                backlog += [(None, (lambda st=st: v_chunk(st)), 640)
                            for st in range(0, 8)]
            if i == 2:
                backlog += [(None, (lambda st=st: v_chunk(st)), 640)
                            for st in range(8, 16)]
            if 2 <= i < 6:
                backlog.append((f"kd1{i - 2}", (lambda c=i - 2: (
                    k_chunk(1, c), dup_k(2, c), dup_k(3, c))), 1280))
            if i == 5:
                backlog.append(("qd10", (lambda: (
                    q_chunk(1, 0), dup_q(2, 0), dup_q(3, 0))), 1280))
            if i == 6:
                backlog.append(("qd11", (lambda: (
                    q_chunk(1, 1), dup_q(2, 1), dup_q(3, 1))), 1280))
            if h == 0 and i < 8 and qt + 1 < QT_N:
                q_chunk(0, qt + 1); dup_q(0, qt + 1); dup_q(1, qt + 1)
            if h == 3 and 8 <= i < 13:
                backlog.append((f"qd1{qt + 2}", (lambda qn=qt + 2: (
                    q_chunk(1, qn), dup_q(2, qn), dup_q(3, qn))), 1280))
            for j in av_due(i):
                avu = av_units_for(j)
                # av_qs 433ns, transpose 53, o_proj 427/853 -- rough PE costs
                costs = [433] * 4 + [53] * max(0, min(4, len(avu) - 4)) \
                    + [640] * max(0, len(avu) - 8)
                backlog += [(f"av{j}", u, c) for u, c in zip(avu, costs)]
            # producer guarantees before emitting qk(i): its q-dup bundle,
            # and the AV group whose exps-pool slot phase i reuses
            drain_until(f"av{i - 5}")
            drain_until(f"av{i - 4}")
            if i == 8:
                drain_until("kd13")
                drain_until("qd10")
            if i >= 10 and h == 2:
                drain_until(f"qd1{qt}")
            ensure_tiles(qt)
            e[groups[i]], qk = qk_units(qt, h, last=(i == len(groups) - 1))
            budgets = [620.0 if i < 12 else 900.0] * len(qk)
            weave(qk, budgets)
            if dbg is not None and i - AV_LAG == 1:
                nc.sync.dma_start(dbg["oN0"], oN[0][:])
        # drain the backlog and the last group
        tail = [u for _, u, _ in backlog]
        backlog.clear()
        tail += av_units_for(15)
        for u in tail:
            u()
        if dbg is not None:
            nc.sync.dma_start(dbg["kstg"], kstg[0][:])
            nc.sync.dma_start(dbg["qstg"], qstg[0][:])
            nc.sync.dma_start(dbg["v"], v_sb[:])
            for hh in range(HPC):
                nc.sync.dma_start(dbg["ktile"][:, hh], ktile[hh][:])
                nc.sync.dma_start(dbg["qtile"][:, hh], qtile[hh][:])


def _fp8_split(a):
    f8 = mybir.dt.np(mybir.dt.float8e4)
    hi = np.ascontiguousarray(a).astype(f8)
    lo = np.ascontiguousarray(a - hi.astype(np.float32)).astype(f8)
    return hi, lo


def _prep_inputs(x, Wq, bq, Wk, bk, Wv, bv, Wo, bo):
    bf = ml_dtypes.bfloat16
    x = np.asarray(x, np.float32)
    in_maps = []
    for c in range(N_CORES):
        b, hq = c // 4, c % 4
        cs = slice(hq * HD, (hq + 1) * HD)
        xa, xb = _fp8_split(XS * x[b].T)
        # x at XS=0.3003, W at x16: K/Q PSUM = sqrt(G2)*(xW+b), V = 4.8*xWv
        wqa, wqb = _fp8_split(16.0 * np.asarray(Wq, np.float32)[:, cs])
        wka, wkb = _fp8_split(16.0 * np.asarray(Wk, np.float32)[:, cs])
        wva, wvb = _fp8_split(16.0 * np.asarray(Wv, np.float32)[:, cs])
        in_maps.append(
            {
                "xTa": xa, "xTb": xb,
                "wq": np.ascontiguousarray(np.stack([wqa, wqb], axis=1)),
                "wk": np.ascontiguousarray(np.stack([wka, wkb], axis=1)),
                "wv": np.ascontiguousarray(np.stack([wva, wvb], axis=1)),
                "wo": np.ascontiguousarray(
                    np.asarray(Wo, np.float32)[cs, :] / (16.0 * XS)
                ).astype(bf),
                "bq": np.ascontiguousarray(
                    16.0 * XS * np.asarray(bq, np.float32)[cs]),
                "bk": np.ascontiguousarray(
                    16.0 * XS * np.asarray(bk, np.float32)[cs]),
            }
        )
    return in_maps


def get_program():
    if "nc" not in _CACHE:
        _CACHE["nc"] = _build_program()
    return _CACHE["nc"]


def run(inputs, **kw):
    nc = get_program()
    in_maps = _prep_inputs(**inputs)
    res = run_bass_kernel_spmd(nc, in_maps, core_ids=list(range(N_CORES)), **kw)
    # final bias: bo + bv @ Wo (bv folds out of attention since softmax rows
    # sum to 1), computed in fp32 on host
    bias = np.asarray(inputs["bo"], np.float32) + np.asarray(
        inputs["bv"], np.float32
    ) @ np.asarray(inputs["Wo"], np.float32)
    out = np.empty((2, S, D), np.float32)
    for b in range(2):
        acc = res.results[4 * b]["y"].astype(np.float32).copy()
        for i in range(1, 4):
            acc += res.results[4 * b + i]["y"]
        out[b] = acc + bias
    return out, res


def kernel(**inputs):
    out, _ = run(inputs)
    return out


# revision 33
# speedup vs baseline: 1.0901x; 1.0029x over previous
"""Trainium2 Bass kernel for CustomMultiHeadAttention.

Problem: x[2,2048,1024], 16 heads, Dh=64. y = MHA(x) with Q/K/V/O projections.

Sharding (8 cores, no collectives):
  core c -> batch b = c//4, head-quarter hq = c%4 (4 heads, 256 model cols).
  Each core computes Q,K,V projections for its 4 heads over the FULL batch
  sequence, attention for those heads, and a PARTIAL o_proj (its 256 rows of
  Wo). The host sums the 4 bf16 partials per batch in fp32 and adds
  bo + bv@Wo (bv folds out of attention since softmax rows sum to 1).

Projections (fp8 DoubleRow residual):
  x ships as fp8e4m3 (hi, lo) pairs scaled by 0.3003 and Wq/Wk/Wv x16, so
  Q/K land in PSUM already at the exp2-friendly scale g = sqrt(16*log2 e)
  = 4.8045: S_psum = g^2 qk = 128*log2(e)*(qk/8) -- the bf16 exponent-field
  value of exp(qk/8) directly. Each projection sums the 3 fp8 cross terms
  with DoubleRow matmuls (2 k-tiles per instruction, 0.5 cycles/row).

QK at fp8-DoubleRow rate (the 4-cross-term packing):
  K and Q are re-quantized on-chip to fp8 (hi, lo) residual pairs by DVE.
  K tile [128, 2, S]: partitions 0-63 hold head dims 0-63 as interleaved
  (hi, lo) pairs; partitions 64-127 duplicate them (SBUF->SBUF DMA).
  Q tile [128, S]: partitions 0-63 = Q_hi, 64-127 = Q_lo of the same dims;
  the matmul rhs broadcasts the pair axis with a 0-stride AP. One DoubleRow
  matmul then contracts all 4 cross terms (Khi+Klo)(Qhi+Qlo) = full-precision
  K.Q at 0.5 cycles/row -- 2x the bf16 rate, fp8-pair (~bf16) accuracy.

Softmax exp split across ACT and DVE:
  ACT chunks: activation(Exp, scale=ln2/128) on [P,2,512] PSUM chunks.
  DVE chunks: custom 8-stage DVE op EXP2C_ANT computes the bf16 BITS of
  2^(y/128) as int16: y'=y-64; k=((y'+M)-M) with M=1.5*2^30 floors y to a
  multiple of 128; i = y' + 0.0027*(y'-k)^2 + D. Round-to-nearest int16
  convert + bf16 bitcast yield exp with ~0.25% RMS error whose constant
  factor cancels in the softmax normalization. This offloads a tunable
  fraction of the exp stream from the saturated ACT engine onto DVE.

Attention dataflow (as baseline):
  S^T per (head, kt, qt512) in PSUM; P^T -> exps pool bf16;
  AV with q on partitions + ones column for sumexp; DVE recip+scale;
  PE transpose; o_proj bf16 with partial-Wo; host reduce.

PSUM (8 banks): psQ 2x[P,2,512] QK/exp staging, ps 2x[P,512]
(proj/o_proj/transpose/warmup ring), av 2x[P,512] (AV accumulators).
"""

import os

import numpy as np
import ml_dtypes

import concourse.bass as bass
import concourse.mybir as mybir
import concourse.tile as tile
from concourse import bacc
from concourse.bass_utils import run_bass_kernel_spmd
from concourse.masks import make_identity

# ---- custom DVE op: bf16-bitcast exp2 with quadratic mantissa correction ----
from concourse import dve_ops as _DO
from concourse.dve_spec import Spec, Src0, Src1, C0, C1, C2, lower as _dve_lower, sq as _dve_sq
from concourse.dve_uop import DveOpSpec as _DveOpSpec

EXP2_MAGIC = 1.5 * (2.0 ** 30)
EXP2_C2 = 0.0027
EXP2_D = 16309.036


def _exp2_ref(in0, in1, s0, s1, imm2):
    y1 = (in0.astype(np.float32) - np.float32(s0)).astype(np.float32)
    z = (y1 + np.float32(s1)).astype(np.float32)
    k = (z - np.float32(s1)).astype(np.float32)
    t = (y1 - k).astype(np.float32)
    return ((t * t).astype(np.float32) * np.float32(imm2) + y1).astype(
        np.float32
    ) + in1


def _register_exp2_op():
    name = "EXP2C_ANT"
    for o in _DO.OPS:
        if o.name == name:
            return o
    _y1 = Src0 - C0
    _z = _y1 + C1
    _k = _z - C1
    _t = _y1 - _k
    spec = Spec(body=(_dve_sq(_t) * C2 + _y1) + Src1, reference=_exp2_ref)
    row = _DO._CUSTOM_DVE_ROW_BASE + len(_DO.OPS)
    assert row < 0x20
    shas = {}
    for ver in ("v3", "v4"):
        s = _DveOpSpec(name=name, opcode=row, uops=_dve_lower(spec, ver=ver), rd1_en=True)
        shas[ver] = s.sha(ver)
    op = _DO.DveOp(name, spec, subdim=False, uops_sha=shas)
    _DO.OPS.append(op)
    _DO._SUB_OPCODE_FOR_NAME[name] = row
    _DO.CUSTOM_DVE_SPECS[name] = spec
    return op


EXP2_OP = _register_exp2_op()

P = 128
S = 2048
D = 1024
H = 16
DH = 64
HPC = 4          # heads per core
HD = HPC * DH    # 256 model cols per core
KO = D // P      # 8 contraction subtiles for the projections
KT_N = S // P    # 16 key tiles
QT = 512         # query tile
QT_N = S // QT   # 4 query tiles
N_CORES = 8

BF16 = mybir.dt.bfloat16
FP8 = mybir.dt.float8e4
F32 = mybir.dt.float32
U16 = mybir.dt.uint16
DR = mybir.MatmulPerfMode.DoubleRow
EXP = mybir.ActivationFunctionType.Exp
ADD = mybir.AluOpType.add
SUB = mybir.AluOpType.subtract

# scale split: x staged at XS, Wq/Wk at 16 -> S_psum = 128*log2(e)*S_true/8
G2 = 128.0 * np.log2(np.e) * 0.125          # 23.0831...
XS = np.sqrt(G2) / 16.0                     # 0.300281...
ACT_EXP_SCALE = float(np.log(2.0) / 128.0)  # exp(S_psum * this) = exp(qk/8)

# which psQ chunks of each qk phase go to the DVE exp2 path (of ~8)
DVE_EXP_IDX = frozenset((2, 6))

_CACHE = {}
KDBG = os.environ.get("KDBG") == "1"  # debug: dump intermediates to DRAM


def _build_program():
    nc = bacc.Bacc(
        "TRN2",
        target_bir_lowering=False,
        debug=False,
        enable_asserts=False,
        num_devices=N_CORES,
    )
    xT = [nc.dram_tensor(f"xT{t}", [D, S], FP8, kind="ExternalInput").ap()
          for t in "ab"]
    wq = nc.dram_tensor("wq", [D, 2, HD], FP8, kind="ExternalInput").ap()
    wk = nc.dram_tensor("wk", [D, 2, HD], FP8, kind="ExternalInput").ap()
    wv = nc.dram_tensor("wv", [D, 2, HD], FP8, kind="ExternalInput").ap()
    wo = nc.dram_tensor("wo", [HD, D], BF16, kind="ExternalInput").ap()
    bq = nc.dram_tensor("bq", [HD], F32, kind="ExternalInput").ap()
    bk = nc.dram_tensor("bk", [HD], F32, kind="ExternalInput").ap()
    y = nc.dram_tensor("y", [S, D], BF16, kind="ExternalOutput").ap()
    dbg = None
    if KDBG:
        dbg = {
            "kstg": nc.dram_tensor("dbg_kstg", [P, 2, S], FP8, kind="ExternalOutput").ap(),
            "qstg": nc.dram_tensor("dbg_qstg", [P, 2, S], FP8, kind="ExternalOutput").ap(),
            "ktile": nc.dram_tensor("dbg_ktile", [P, HPC, 2, S], FP8, kind="ExternalOutput").ap(),
            "qtile": nc.dram_tensor("dbg_qtile", [P, HPC, S], FP8, kind="ExternalOutput").ap(),
            "v": nc.dram_tensor("dbg_v", [P, KT_N, HPC, DH + 1], BF16, kind="ExternalOutput").ap(),
            "exp00": nc.dram_tensor("dbg_exp00", [P, KT_N, QT], BF16, kind="ExternalOutput").ap(),
            "oN0": nc.dram_tensor("dbg_oN0", [P, 4, HPC, DH], BF16, kind="ExternalOutput").ap(),
        }

    with tile.TileContext(nc) as tc:
        _body(tc, y, xT, wq, wk, wv, wo, bq, bk, dbg)
    nc.compile()
    return nc


def _body(tc, y, xT, wq, wk, wv, wo, bq, bk, dbg=None):
    nc = tc.nc
    with (
        tc.tile_pool(name="const", bufs=1) as const,
        tc.tile_pool(name="big", bufs=1) as big,
        tc.tile_pool(name="exps", bufs=5) as exps,
        tc.tile_pool(name="onp", bufs=4) as onp,
        tc.tile_pool(name="rsp", bufs=4) as rsp,
        tc.tile_pool(name="yst", bufs=3) as yst,
        tc.tile_pool(name="psQ", bufs=2, space="PSUM") as psQ,
        tc.tile_pool(name="psA", bufs=2, space="PSUM") as psA,
    ):
        # ---- constant / persistent tiles (DMA order = HWDGE issue order;
        # wk + the first xT chunks gate the first QK, so they go first) ----
        wk_sb = const.tile([P, KO, 2, HD], FP8, tag="wk")
        wq_sb = const.tile([P, KO, 2, HD], FP8, tag="wq")
        xT_sb = [big.tile([P, KO, S], FP8, tag=f"xT{t}", name=f"xT{t}")
                 for t in "ab"]
        wk_r = wk.rearrange("(ko p) t m -> p ko t m", p=P)
        wq_r = wq.rearrange("(ko p) t m -> p ko t m", p=P)
        x_r = [t.rearrange("(ko p) s -> p ko s", p=P) for t in xT]
        for half in range(2):
            ks = slice(4 * half, 4 * half + 4)
            nc.sync.dma_start(wk_sb[:, ks], wk_r[:, ks])
            nc.sync.dma_start(xT_sb[0][:, ks, 0:QT], x_r[0][:, ks, 0:QT])
            nc.sync.dma_start(wq_sb[:, ks], wq_r[:, ks])
            nc.sync.dma_start(xT_sb[1][:, ks, 0:QT], x_r[1][:, ks, 0:QT])

        bk_sb = const.tile([P, HD // P], F32, tag="bk")
        nc.sync.dma_start(bk_sb[:], bk.rearrange("(o p) -> p o", p=P))
        bq_sb = const.tile([P, HD // P], F32, tag="bq")
        nc.sync.dma_start(bq_sb[:], bq.rearrange("(o p) -> p o", p=P))

        for c in range(1, QT_N):
            cs = slice(c * QT, (c + 1) * QT)
            for tt in range(2):
                nc.sync.dma_start(xT_sb[tt][:, :, cs], x_r[tt][:, :, cs])

        # wv / wo are DMA'd lazily from the startup sequence so the K/Q dup
        # DMAs (which gate the first QK phases) don't queue behind them
        wv_sb = const.tile([P, KO, 2, HD], FP8, tag="wv")
        wo_sb = const.tile([P, HD // P, D], BF16, tag="wo")

        def load_wv():
            nc.sync.dma_start(wv_sb[:], wv.rearrange("(ko p) t m -> p ko t m", p=P))

        def load_wo():
            nc.sync.dma_start(wo_sb[:], wo.rearrange("(ks p) n -> p ks n", p=P))

        # V with a trailing ones column: col 64 = 1 so the AV matmul emits
        # the sumexp in out column 64.
        v_sb = big.tile([P, KT_N, HPC, DH + 1], BF16, tag="v")
        nc.vector.memset(v_sb[:, :, :, DH : DH + 1], 1.0)

        # K/Q fp8 residual-pair stores.
        # kstg/qstg: undup'd staging [dims, (hi,lo), S] per mt, written by DVE
        # from the projection PSUM. ktile[h]: partitions 0-63 = head dims 0-63
        # as (hi,lo) pairs, 64-127 duplicate. qtile[h]: 0-63 = hi, 64-127 = lo.
        kstg = [big.tile([P, 2, S], FP8, tag=f"kstg{mt}", name=f"kstg{mt}")
                for mt in range(2)]
        qstg = [big.tile([P, 2, S], FP8, tag=f"qstg{mt}", name=f"qstg{mt}")
                for mt in range(2)]
        ktile = [big.tile([P, 2, S], FP8, tag=f"ktile{h}", name=f"ktile{h}")
                 for h in range(HPC)]
        qtile = [big.tile([P, S], FP8, tag=f"qtile{h}", name=f"qtile{h}")
                 for h in range(HPC)]

        oT_sb = big.tile([P, HD // P, S], BF16, tag="oT")

        # D-constant tile for the custom exp2 op's final add
        dtile = const.tile([P, 2, QT], F32, tag="dtile")
        nc.vector.memset(dtile[:], EXP2_D)

        ident = const.tile([P, P], BF16, tag="ident")
        make_identity(nc, ident[:])

        # ---- PE warmup: dummy matmuls so the p-state ramp is underway ----
        wu = const.tile([P, QT], BF16, tag="wu")
        nc.vector.memset(wu[:], 0.0)
        for i in range(5):
            pwu = psA.tile([P, QT], F32, tag="ps", name="pwu")
            nc.tensor.matmul(
                pwu[:], lhsT=wu[:, 0:P], rhs=wu[:], start=True, stop=True
            )

        # ---- projections: 1-bank [P,512] chunks through the ps ring.
        # fp8 DoubleRow residual: 3 cross terms (xa*wa + xa*wb + xb*wa),
        # each contracting 2 k-tiles per matmul -> 12 matmuls a chunk ----
        TERMS = ((0, 0), (0, 1), (1, 0))

        def proj_mms(out, w_sb, x_pair, mslc, nslc):
            n = 0
            for xt_i, w_i in TERMS:
                for j in range(KO // 2):
                    n += 1
                    nc.tensor.matmul(
                        out,
                        lhsT=w_sb[:, 2 * j : 2 * j + 2, w_i, mslc],
                        rhs=x_pair[xt_i][:, 2 * j : 2 * j + 2, nslc],
                        start=(n == 1),
                        stop=(n == 3 * (KO // 2)),
                        perf_mode=DR,
                    )

        def stage_pair(stg, psum, bias_ap, cols):
            # hi = fp8(psum + bias); lo = fp8((psum + bias) - hi)
            nc.vector.tensor_scalar_add(stg[:, 0, cols], psum, bias_ap)
            nc.vector.scalar_tensor_tensor(
                out=stg[:, 1, cols], in0=psum, scalar=bias_ap,
                in1=stg[:, 0, cols], op0=ADD, op1=SUB,
            )

        def k_chunk(mt, nt):
            pk = psA.tile([P, QT], F32, tag="ps", name="pk")
            proj_mms(pk[:], wk_sb, xT_sb, slice(mt * P, (mt + 1) * P),
                     slice(nt * QT, (nt + 1) * QT))
            stage_pair(kstg[mt], pk[:], bk_sb[:, mt : mt + 1],
                       slice(nt * QT, (nt + 1) * QT))

        def q_chunk(mt, qt):
            qs = slice(qt * QT, (qt + 1) * QT)
            pq = psA.tile([P, QT], F32, tag="ps", name="pq")
            proj_mms(pq[:], wq_sb, xT_sb, slice(mt * P, (mt + 1) * P), qs)
            stage_pair(qstg[mt], pq[:], bq_sb[:, mt : mt + 1], qs)

        def dup_k(h, part=None):
            # ktile[h][0:64] and [64:128] <- kstg[mt][hb:hb+64]
            mt, hb = h // 2, (h % 2) * DH
            cols = slice(None) if part is None else slice(part * QT, (part + 1) * QT)
            src = kstg[mt][hb : hb + DH, :, cols]
            nc.sync.dma_start(ktile[h][0:DH, :, cols], src)
            nc.sync.dma_start(ktile[h][DH:P, :, cols], src)

        def dup_q(h, qt):
            # qtile[h][0:64] <- hi, [64:128] <- lo
            mt, hb = h // 2, (h % 2) * DH
            cols = slice(qt * QT, (qt + 1) * QT)
            nc.sync.dma_start(qtile[h][0:DH, cols], qstg[mt][hb : hb + DH, 0, cols])
            nc.sync.dma_start(qtile[h][DH:P, cols], qstg[mt][hb : hb + DH, 1, cols])

        def v_chunk(st):
            pv = psA.tile([P, QT], F32, tag="ps", name="pv")
            n = 0
            for xt_i, w_i in TERMS:
                for j in range(KO // 2):
                    n += 1
                    nc.tensor.matmul(
                        pv[:, 0:HD],
                        lhsT=xT_sb[xt_i][:, 2 * j : 2 * j + 2, st * P : (st + 1) * P],
                        rhs=wv_sb[:, 2 * j : 2 * j + 2, w_i, :],
                        start=(n == 1),
                        stop=(n == 3 * (KO // 2)),
                        perf_mode=DR,
                    )
            nc.vector.tensor_copy(
                out=v_sb[:, st, :, 0:DH],
                in_=pv[:, 0:HD].rearrange("p (h c) -> p h c", h=HPC),
            )

        # ---- attention ----
        def qk_chunk_act(exp_t, h, rhs, kt, w):
            """One ACT psQ chunk of a QK phase: w DoubleRow matmuls + exp.
            The psQ ring carries ONLY ACT chunks, so the exp(c)->QK(c+2)
            chain latency always hides behind exp(c+1) and ACT is gapless."""
            pqk = psQ.tile([P, 2, QT], F32, tag="psq", name="pqk")
            for j in range(w):
                nc.tensor.matmul(
                    pqk[:, j, :],
                    lhsT=ktile[h][:, :, (kt + j) * P : (kt + j + 1) * P],
                    rhs=rhs,
                    start=True,
                    stop=True,
                    perf_mode=DR,
                )
            nc.scalar.activation(
                exp_t[:, kt : kt + w, :], pqk[:, 0:w, :], EXP,
                scale=ACT_EXP_SCALE,
            )

        def qk_chunk_dve(exp_t, h, rhs, kt):
            """One DVE single-kt chunk, staged through the av ring (1 bank)
            so it never blocks the ACT psQ ring."""
            pq1 = psA.tile([P, QT], F32, tag="av", name="dq")
            nc.tensor.matmul(
                pq1[:],
                lhsT=ktile[h][:, :, kt * P : (kt + 1) * P],
                rhs=rhs,
                start=True,
                stop=True,
                perf_mode=DR,
            )
            nc.vector._custom_dve(
                EXP2_OP,
                out=exp_t[:, kt, :].bitcast(U16),
                in0=pq1[:],
                in1=dtile[:, 0, :],
                s0=64.0, s1=EXP2_MAGIC, imm2=EXP2_C2,
            )

        # per-phase chunk patterns: (width, is_dve). DVE singles carry 4 of
        # the 16 kt (f=0.25); the last phases shift more onto DVE so the
        # tail exp drains fast.
        PAT_STD = [(2, 0), (2, 0), (1, 1), (2, 0), (1, 1),
                   (2, 0), (2, 0), (1, 1), (2, 0), (1, 1)]
        PAT_FIRST = [(1, 0), (1, 0), (2, 0), (1, 1), (2, 0), (1, 1),
                     (2, 0), (2, 0), (1, 1), (2, 0), (1, 1)]
        PAT_LAST = [(2, 0), (1, 1), (2, 0), (1, 1), (2, 0), (1, 1),
                    (2, 0), (1, 1), (2, 0), (1, 1), (1, 0)]

        def qk_units(qt, h, last=False, first=False):
            """Closures for one (query-tile, head) QK+exp phase, one per
            chunk, so other PE work can be woven between chunks (the in-order
            PE queue would otherwise head-block on the psum rings)."""
            qs = slice(qt * QT, (qt + 1) * QT)
            exp_t = exps.tile([P, KT_N, QT], BF16, tag="exps", name=f"exp{qt}{h}")
            rhs = qtile[h][:, qs].unsqueeze(1).broadcast_to([P, 2, QT])
            pat = PAT_LAST if last else (PAT_FIRST if first else PAT_STD)
            units = []
            kt = 0
            for w, dve in pat:
                if dve:
                    units.append(lambda kt=kt: qk_chunk_dve(exp_t, h, rhs, kt))
                else:
                    units.append(lambda kt=kt, w=w: qk_chunk_act(exp_t, h, rhs, kt, w))
                kt += w
            return exp_t, units

        def transpose_qs(qt, hp, qs, oN, on_act=False):
            # oN [q, 2 heads x 64] -> oT [128 d, q] via a PE transpose
            cols = slice(qt * QT + qs * P, qt * QT + (qs + 1) * P)
            tp = psA.tile([P, P], BF16, tag="ps", name="tp")
            nc.tensor.transpose(tp[:], oN[:, qs, 2 * hp : 2 * hp + 2, :], ident[:])
            if on_act:
                nc.scalar.activation(oT_sb[:, hp, cols], tp[:],
                                     mybir.ActivationFunctionType.Copy)
            else:
                nc.vector.tensor_copy(out=oT_sb[:, hp, cols], in_=tp[:])

        def o_proj_qs(qt, qs, copy_act=False):
            rows = slice(qt * QT + qs * P, qt * QT + (qs + 1) * P)
            for nt in range(D // QT):
                py = psA.tile([P, QT], F32, tag="ps", name="py")
                for ks in range(HD // P):
                    nc.tensor.matmul(
                        py[:],
                        lhsT=oT_sb[:, ks, rows],
                        rhs=wo_sb[:, ks, nt * QT : (nt + 1) * QT],
                        start=(ks == 0),
                        stop=(ks == HD // P - 1),
                    )
                yt = yst.tile([P, QT], BF16, tag="yt", name="yt")
                if copy_act:
                    nc.scalar.activation(yt[:], py[:],
                                         mybir.ActivationFunctionType.Copy)
                else:
                    nc.vector.tensor_copy(out=yt[:], in_=py[:])
                nc.sync.dma_start(y[rows, nt * QT : (nt + 1) * QT], yt[:])

        def o_proj_tail_qs(qt, qs):
            # last-qt variant: QK is done, so the freed psQ ring supplies a
            # 2-bank tile per qs; the PSUM->SBUF copy goes to ACT (idle after
            # the last exp) or DVE, and both nt halves leave in one DMA.
            rows = slice(qt * QT + qs * P, qt * QT + (qs + 1) * P)
            py = psQ.tile([P, 2, QT], F32, tag="psq", name="py")
            yt = yst.tile([P, 2, QT], BF16, tag="yt", name="yt")
            for nt in range(D // QT):
                for ks in range(HD // P):
                    nc.tensor.matmul(
                        py[:, nt, :],
                        lhsT=oT_sb[:, ks, rows],
                        rhs=wo_sb[:, ks, nt * QT : (nt + 1) * QT],
                        start=(ks == 0),
                        stop=(ks == HD // P - 1),
                    )
            nc.scalar.activation(
                yt[:, 0, :], py[:, 0, :], mybir.ActivationFunctionType.Copy
            )
            nc.vector.tensor_copy(out=yt[:, 1, :], in_=py[:, 1, :])
            nc.sync.dma_start(y[rows, :], yt.rearrange("p a b -> p (a b)"))

        def av_qs(qt, h, exp_t, oN, rsum, qs, oproj_inline=False):
            """One qs128 slice of AV: accumulate out[q,65] over kt into a full
            PSUM bank (start=True resets the bank), then recip/normalize."""
            tag = "ps" if (oproj_inline and qs >= 2) else "av"
            av = psA.tile([P, QT], F32, tag=tag, name="av")
            for kt in range(KT_N):
                nc.tensor.matmul(
                    av[:, 0 : DH + 1],
                    lhsT=exp_t[:, kt, qs * P : (qs + 1) * P],
                    rhs=v_sb[:, kt, h, :],
                    start=(kt == 0),
                    stop=(kt == KT_N - 1),
                )
            nc.vector.reciprocal(
                rsum[:, h, qs : qs + 1], av[:, DH : DH + 1]
            )
            nc.vector.tensor_scalar_mul(
                oN[:, qs, h, :], av[:, 0:DH], rsum[:, h, qs : qs + 1]
            )

        def av_units(qt, h, exp_t, oN, rsum, oproj_inline=False):
            """Closures for one AV group: 4 av_qs, then (for pair-completing
            heads) transposes, then o_proj. Transposes come after the whole
            AV block: their matmuls wait on the DVE norms, and emitting them
            mid-block head-blocks the in-order PE queue behind that wait."""
            units = [
                lambda qs=qs: av_qs(qt, h, exp_t, oN, rsum, qs, oproj_inline)
                for qs in range(QT // P)
            ]
            if h == 1:
                units += [lambda qs=qs: transpose_qs(qt, 0, qs, oN)
                          for qs in range(QT // P)]
            elif h == 3:
                late = qt >= 2
                trs = [lambda qs=qs, a=late: transpose_qs(qt, 1, qs, oN, a)
                       for qs in range(QT // P)]
                if oproj_inline:
                    # tail: interleave av/transpose/o_proj per qs so the
                    # final chain pipelines instead of running in strata
                    opj = [lambda qs=qs: o_proj_tail_qs(qt, qs)
                           for qs in range(QT // P)]
                    units = [units[0], units[1], units[2], trs[0], units[3],
                             trs[1], opj[0], trs[2], opj[1], trs[3],
                             opj[2], opj[3]]
                elif qt != QT_N - 1:
                    units += trs + [lambda qs=qs, a=late: o_proj_qs(qt, qs, a)
                                    for qs in range(QT // P)]
                    trs = None
                if trs is not None and not oproj_inline:
                    units += trs
            return units

        # ---- emission schedule: two head-pair passes (all qt with heads
        # 0/1, then heads 2/3). qk phase i is emitted AT iter i, chunk-woven
        # with AV of phase i-3 and the projection/dup work scheduled for the
        # iter, so the in-order PE queue never head-blocks on the psQ ring
        # (QK at fp8-DR rate is ~5x faster than the ACT/DVE exp drain). ----
        groups = [(qt, h) for hp in range(2) for qt in range(QT_N)
                  for h in (2 * hp, 2 * hp + 1)]
        AV_LAG = 2

        oN, rsum = {}, {}

        def ensure_tiles(qt):
            if qt not in oN:
                oN[qt] = onp.tile(
                    [P, QT // P, HPC, DH], BF16, tag="oN", name=f"oN{qt}"
                )
                rsum[qt] = rsp.tile(
                    [P, HPC, QT // P], F32, tag="rs", name=f"rs{qt}"
                )

        e = {}

        def av_units_for(j):
            qtj, hj = groups[j]
            return av_units(
                qtj, hj, e.pop(groups[j]), oN[qtj], rsum[qtj],
                oproj_inline=(j == len(groups) - 1),
            )

        backlog = []  # ordered (key, fn, pe_ns) filler units; drained lazily

        def drain_until(key):
            # force-emit backlog items up to and including `key` (producer
            # guarantees: a consumer may only be emitted after its producer)
            if not any(k == key for k, _, _ in backlog):
                return
            while backlog:
                k, fn, _ = backlog.pop(0)
                fn()
                if k == key:
                    return

        def weave(qk, budgets):
            # prime both psQ slots, then pace fillers by estimated PE time.
            # budgets[i] is the filler time allowed before qk chunk i: the
            # exp(c)->QK(c+2) chain hides behind exp(c+1) on the other psQ
            # slot, EXCEPT right after a DVE chunk (ACT is then waiting
            # immediately), so those positions get zero budget.
            qk[0](); qk[1]()
            for u, b in zip(qk[2:], budgets[2:]):
                cover = 0.0
                while cover < b and backlog:
                    _, fn, ns = backlog.pop(0)
                    fn()
                    cover += ns
                u()

        # --- startup: x chunks land one by one (~3.2us apart); emit every
        # projection chunk (both mt, + fp8 staging + dup DMAs) as its x chunk
        # arrives, with the part-gated first QK phase woven in so ACT starts
        # early. Every unit here is emitted strictly after its producers
        # (Tile deps follow program order). ---
        # Keep pre-phase-1 PE work UNDER the ~13us x-DMA gate: only K mt0,
        # q(0,0), and the part-gated first phase. Everything else (K mt1, V,
        # q mt1) defers into the backlog.
        ensure_tiles(0)
        k_chunk(0, 0); dup_k(0, 0); dup_k(1, 0)
        q_chunk(0, 0); dup_q(0, 0); dup_q(1, 0)
        e[groups[0]], qk0 = qk_units(0, 0, first=True)
        e[groups[1]], qk1 = qk_units(0, 1)
        # both head-0 and head-1 phases are part-gated through the startup,
        # doubling the early exp supply while x chunks land
        qk0[0](); qk0[1](); qk0[2](); qk1[0](); qk1[1]()   # part 0
        k_chunk(0, 1); dup_k(0, 1); dup_k(1, 1)
        load_wv()
        qk0[3](); qk1[2](); qk0[4](); qk1[3](); qk0[5](); qk1[4]()  # part 1
        k_chunk(0, 2); dup_k(0, 2); dup_k(1, 2)
        qk0[6](); qk1[5](); qk0[7](); qk1[6]()        # part 2
        k_chunk(0, 3); dup_k(0, 3); dup_k(1, 3)
        load_wo()
        qk0[8](); qk1[7](); qk0[9](); qk0[10](); qk1[8](); qk1[9]()  # part 3
        q_chunk(0, 1); dup_q(0, 1); dup_q(1, 1)
        if dbg is not None:
            nc.sync.dma_start(dbg["exp00"], e[groups[0]][:])

        # --- iters 1..15: qk(i) woven with the filler backlog (av(i-3),
        # transposes, o_proj, remaining projections, v chunks) ---
        def av_due(i):
            # deferred AV ramp: the first groups wait until the x-DMA-gated
            # startup projections/v-chunks have drained, then settle to lag
            # 3, and the last iters tighten to lag 2 so the tail is short
            return {13: [10, 11], 14: [12, 13], 15: [14]}.get(
                i, [i - 3] if 3 <= i <= 12 else [])

        for i in range(2, len(groups)):
            qt, h = groups[i]
            if i == 2:
                backlog += [(None, (lambda st=st: v_chunk(st)), 640)
                            for st in range(0, 16)]
            if 2 <= i < 6:
                backlog.append((f"kd1{i - 2}", (lambda c=i - 2: (
                    k_chunk(1, c), dup_k(2, c), dup_k(3, c))), 1280))
            if i == 5:
                backlog.append(("qd10", (lambda: (
                    q_chunk(1, 0), dup_q(2, 0), dup_q(3, 0))), 1280))
            if i == 6:
                backlog.append(("qd11", (lambda: (
                    q_chunk(1, 1), dup_q(2, 1), dup_q(3, 1))), 1280))
            if h == 0 and i < 8 and qt + 1 < QT_N:
                q_chunk(0, qt + 1); dup_q(0, qt + 1); dup_q(1, qt + 1)
            if h == 3 and 8 <= i < 13:
                backlog.append((f"qd1{qt + 2}", (lambda qn=qt + 2: (
                    q_chunk(1, qn), dup_q(2, qn), dup_q(3, qn))), 1280))
            for j in av_due(i):
                avu = av_units_for(j)
                # av_qs 433ns, transpose 53, o_proj 427/853 -- rough PE costs
                costs = [433] * 4 + [53] * max(0, min(4, len(avu) - 4)) \
                    + [640] * max(0, len(avu) - 8)
                backlog += [(f"av{j}", u, c) for u, c in zip(avu, costs)]
            # producer guarantees before emitting qk(i): its q-dup bundle,
            # and the AV group whose exps-pool slot phase i reuses
            drain_until(f"av{i - 5}")
            drain_until(f"av{i - 4}")
            if i == 8:
                drain_until("kd13")
                drain_until("qd10")
            if i >= 10 and h == 2:
                drain_until(f"qd1{qt}")
            ensure_tiles(qt)
            e[groups[i]], qk = qk_units(qt, h, last=(i == len(groups) - 1))
            budgets = [620.0 if i < 12 else 900.0] * len(qk)
            weave(qk, budgets)
            if dbg is not None and i - AV_LAG == 1:
                nc.sync.dma_start(dbg["oN0"], oN[0][:])
        # drain the backlog and the last group
        tail = [u for _, u, _ in backlog]
        backlog.clear()
        tail += av_units_for(15)
        for u in tail:
            u()
        if dbg is not None:
            nc.sync.dma_start(dbg["kstg"], kstg[0][:])
            nc.sync.dma_start(dbg["qstg"], qstg[0][:])
            nc.sync.dma_start(dbg["v"], v_sb[:])
            for hh in range(HPC):
                nc.sync.dma_start(dbg["ktile"][:, hh], ktile[hh][:])
                nc.sync.dma_start(dbg["qtile"][:, hh], qtile[hh][:])


def _fp8_split(a):
    f8 = mybir.dt.np(mybir.dt.float8e4)
    hi = np.ascontiguousarray(a).astype(f8)
    lo = np.ascontiguousarray(a - hi.astype(np.float32)).astype(f8)
    return hi, lo


def _prep_inputs(x, Wq, bq, Wk, bk, Wv, bv, Wo, bo):
    bf = ml_dtypes.bfloat16
    x = np.asarray(x, np.float32)
    in_maps = []
    for c in range(N_CORES):
        b, hq = c // 4, c % 4
        cs = slice(hq * HD, (hq + 1) * HD)
        xa, xb = _fp8_split(XS * x[b].T)
        # x at XS=0.3003, W at x16: K/Q PSUM = sqrt(G2)*(xW+b), V = 4.8*xWv
        wqa, wqb = _fp8_split(16.0 * np.asarray(Wq, np.float32)[:, cs])
        wka, wkb = _fp8_split(16.0 * np.asarray(Wk, np.float32)[:, cs])
        wva, wvb = _fp8_split(16.0 * np.asarray(Wv, np.float32)[:, cs])
        in_maps.append(
            {
                "xTa": xa, "xTb": xb,
                "wq": np.ascontiguousarray(np.stack([wqa, wqb], axis=1)),
                "wk": np.ascontiguousarray(np.stack([wka, wkb], axis=1)),
                "wv": np.ascontiguousarray(np.stack([wva, wvb], axis=1)),
                "wo": np.ascontiguousarray(
                    np.asarray(Wo, np.float32)[cs, :] / (16.0 * XS)
                ).astype(bf),
                "bq": np.ascontiguousarray(
                    16.0 * XS * np.asarray(bq, np.float32)[cs]),
                "bk": np.ascontiguousarray(
                    16.0 * XS * np.asarray(bk, np.float32)[cs]),
            }
        )
    return in_maps


def get_program():
    if "nc" not in _CACHE:
        _CACHE["nc"] = _build_program()
    return _CACHE["nc"]


def run(inputs, **kw):
    nc = get_program()
    in_maps = _prep_inputs(**inputs)
    res = run_bass_kernel_spmd(nc, in_maps, core_ids=list(range(N_CORES)), **kw)
    # final bias: bo + bv @ Wo (bv folds out of attention since softmax rows
    # sum to 1), computed in fp32 on host
    bias = np.asarray(inputs["bo"], np.float32) + np.asarray(
        inputs["bv"], np.float32
    ) @ np.asarray(inputs["Wo"], np.float32)
    out = np.empty((2, S, D), np.float32)
    for b in range(2):
        acc = res.results[4 * b]["y"].astype(np.float32).copy()
        for i in range(1, 4):
            acc += res.results[4 * b + i]["y"]
        out[b] = acc + bias
    return out, res


def kernel(**inputs):
    out, _ = run(inputs)
    return out
